# revision 14
# baseline (speedup 1.0000x reference)
"""EuclideanCodebook forward (eval) on 8 TRN2 NeuronCores.

Data-parallel: the flattened token dim N=131072 is split across 8 cores
(16384 tokens each); the [2048, 64] codebook is replicated.

Host prep: xaugT = [x.T; ones] (65 x N) and embT = [2*e.T; -|e|^2]
(65 x 2048), so one fp32r matmul per (tile, chunk) yields
scores = 2*x@e.T - |e|^2 directly in PSUM (argmax-equivalent to the
reference distance).

Per 128-token tile: 4 matmuls fill a [128, 2048] PSUM tile; vector.max
(top-8) + vector.max_index read the scores straight out of PSUM -> the
argmax index per token, plus the top-2 values (used host-side to patch
near-ties against exact f32 scoring). One indirect DMA then gathers the
winning codebook rows for the quantize output. The perplexity scalar is
derived host-side from the histogram of the device-computed indices.
"""

import sys

import numpy as np

for _p in ("/opt/trn_rl_repo",):
    if _p not in sys.path:
        sys.path.insert(0, _p)

N_CORES = 8
N_TOTAL = 64 * 2048          # flattened tokens
N_CORE = N_TOTAL // N_CORES  # 16384
D = 64
K = 2048
TILE = 128
NTILES = N_CORE // TILE      # 128
CHUNK = 512                  # matmul free-dim limit (f32)
NCHUNK = K // CHUNK          # 4

_cache = {}


def _split_excess_waits(nc, limit=1):
    """The walrus build in this image can't encode instructions carrying
    more than ~2 semaphore waits ("Too many sync wait commands"). Hoist
    excess waits onto NoOp instructions inserted just before the offender."""
    import concourse.mybir as mybir

    counter = [0]
    for f in nc.m.functions:
        for b in f.blocks:
            new_insts = []
            for inst in b.instructions:
                si = inst.sync_info
                if si is not None and si.on_wait and len(si.on_wait) > limit:
                    waits = list(si.on_wait)
                    keep = waits[-limit:]
                    extra = waits[:-limit]
                    for w in extra:
                        counter[0] += 1
                        nop = mybir.InstNoOp(
                            name=f"WSPLIT-{counter[0]}", ins=[], outs=[]
                        )
                        nop.engine = inst.engine
                        nop.sync_info = mybir.SyncInfo(on_wait=[w], on_update=[])
                        new_insts.append(nop)
                    si.on_wait = keep
                new_insts.append(inst)
            b.instructions[:] = new_insts
    return counter[0]


def _build_nc(n_core=N_CORE, k=K, split_waits=True):
    import concourse.bass as bass
    import concourse.mybir as mybir
    from concourse.bass import IndirectOffsetOnAxis
    from concourse.tile import TileContext

    F32R = mybir.dt.float32r
    F32 = mybir.dt.float32
    U32 = mybir.dt.uint32

    ntiles = n_core // TILE
    nchunk = k // CHUNK

    nc = bass.Bass()
    xT_d = nc.declare_dram_parameter("xaugt", [D + 1, n_core], F32R, isOutput=False)
    eT_d = nc.declare_dram_parameter("embt", [D + 1, k], F32R, isOutput=False)
    emb_d = nc.declare_dram_parameter("embed", [k, D], F32, isOutput=False)
    quant_d = nc.declare_dram_parameter("quant", [n_core, D], F32, isOutput=True)
    ind8_d = nc.declare_dram_parameter("ind8", [TILE, 8 * ntiles], U32, isOutput=True)
    m8_d = nc.declare_dram_parameter("m8", [TILE, 8 * ntiles], F32, isOutput=True)

    with TileContext(nc) as tc:
        with (
            tc.tile_pool(name="const", bufs=1) as cpool,
            tc.tile_pool(name="small", bufs=6) as spool,
            tc.tile_pool(name="persist", bufs=1) as ppool,
            tc.tile_pool(name="ps_sc", bufs=2, space="PSUM") as ps_sc,
        ):
            xT_t = cpool.tile([D + 1, n_core], F32R)
            nc.sync.dma_start(xT_t[:], xT_d[:])
            eT_t = cpool.tile([D + 1, k], F32R)
            nc.sync.dma_start(eT_t[:], eT_d[:])

            ind8_all = ppool.tile([TILE, 8 * ntiles], U32)
            m8_all = ppool.tile([TILE, 8 * ntiles], F32)
            gath = ppool.tile([128, (n_core // 128) * D], F32)

            for it in range(ntiles):
                scores = ps_sc.tile([TILE, k], F32, tag="sc")
                for c in range(nchunk):
                    nc.tensor.matmul(
                        scores[:, c * CHUNK : (c + 1) * CHUNK],
                        xT_t[:, it * TILE : (it + 1) * TILE],
                        eT_t[:, c * CHUNK : (c + 1) * CHUNK],
                        start=True,
                        stop=True,
                    )
                m8 = m8_all[:, 8 * it : 8 * it + 8]
                nc.vector.max(m8, scores[:])
                idx8 = ind8_all[:, 8 * it : 8 * it + 8]
                nc.vector.max_index(idx8, m8, scores[:])
                # gather this tile's codebook rows: gath[p, it*D:(it+1)*D]
                # = embed[idx8[p, 0]]  (one row per partition per call)
                nc.gpsimd.indirect_dma_start(
                    out=gath[:, it * D : (it + 1) * D],
                    out_offset=None,
                    in_=emb_d[:],
                    in_offset=IndirectOffsetOnAxis(ap=idx8[:, 0:1], axis=0),
                )
            nc.sync.dma_start(
                quant_d.rearrange("(it pp) e -> pp it e", pp=128),
                gath[:].rearrange("p (c e) -> p c e", c=n_core // 128),
            )
            nc.sync.dma_start(ind8_d[:], ind8_all[:])
            nc.sync.dma_start(m8_d[:], m8_all[:])
    if split_waits:
        _split_excess_waits(nc)
    nc.finalize()
    return nc


def _get_nc():
    if "nc" not in _cache:
        _cache["nc"] = _build_nc()
    return _cache["nc"]


def _host_prep(x, embed):
    x = np.ascontiguousarray(np.asarray(x), dtype=np.float32)
    embed = np.ascontiguousarray(np.asarray(embed), dtype=np.float32)
    flat = x.reshape(-1, D)
    xaugT = np.empty((D + 1, flat.shape[0]), dtype=np.float32)
    xaugT[:D] = flat.T
    xaugT[D] = 1.0
    eT = np.empty((D + 1, K), dtype=np.float32)
    eT[:D] = 2.0 * embed.T
    eT[D] = -(embed * embed).sum(1)
    return flat, xaugT, eT, embed


def _run(x, embed, trace=False):
    from concourse.bass_utils import run_bass_kernel_spmd

    shape = np.asarray(x).shape
    flat, xaugT, eT, embed = _host_prep(x, embed)

    nc = _get_nc()
    in_maps = [
        {
            "xaugt": np.ascontiguousarray(xaugT[:, i * N_CORE : (i + 1) * N_CORE]),
            "embt": eT,
            "embed": embed,
        }
        for i in range(N_CORES)
    ]
    res = run_bass_kernel_spmd(
        nc, in_maps, core_ids=list(range(N_CORES)), trace=trace
    )

    quants = []
    inds = []
    for i, r in enumerate(res.results):
        ind8 = np.asarray(r["ind8"]).reshape(TILE, NTILES, 8)
        m8v = np.asarray(r["m8"]).reshape(TILE, NTILES, 8)
        # ind in token order: token it*128+pp -> ind8[pp, it, 0]
        ind_i = np.ascontiguousarray(ind8[:, :, 0].T).reshape(-1).view(np.int32).copy()
        # host patch for near-ties: where the device's top-2 scores are too
        # close, recompute the argmax in exact f32 (fp32r / ordering noise)
        gap = m8v[:, :, 0] - m8v[:, :, 1]
        pp, it = np.nonzero(gap < 5e-2)
        if len(pp):
            toks = it * TILE + pp
            fl = flat[i * N_CORE + toks]
            sc = 2.0 * fl @ embed.T - (embed * embed).sum(1)[None, :]
            exact = sc.argmax(1).astype(np.int32)
            ind_flat = ind_i.reshape(NTILES, TILE)
            ind_flat[it, pp] = exact
            ind_i = ind_flat.reshape(-1)
        inds.append(ind_i)
        quant_i = np.asarray(r["quant"])
        if len(pp):
            quant_i = quant_i.copy()
            quant_i[toks] = embed[ind_i[toks]]
        quants.append(quant_i)

    quant = np.concatenate(quants, axis=0)
    ind = np.concatenate(inds, axis=0)

    quantize = quant.reshape(shape)
    embed_ind = ind.reshape(shape[:-1])

    counts = np.bincount(ind, minlength=K).astype(np.float32)
    avg_probs = counts / np.float32(N_TOTAL)
    perplexity = np.exp(
        -np.sum(avg_probs * np.log(avg_probs + np.float32(1e-10)))
    ).astype(np.float32)

    return (quantize, embed_ind, perplexity), res


def kernel(x, embed):
    out, _ = _run(x, embed, trace=False)
    return out


# revision 16
# speedup vs baseline: 3394.0939x; 3394.0939x over previous
"""EuclideanCodebook forward (eval) on 8 TRN2 NeuronCores.

Data-parallel: the flattened token dim N=131072 is split across 8 cores
(16384 tokens each); the [2048, 64] codebook is replicated.

Host prep: xaugT = [x.T; ones] (65 x N) and embT = [2*e.T; -|e|^2]
(65 x 2048), so one fp32r matmul per (tile, chunk) yields
scores = 2*x@e.T - |e|^2 directly in PSUM (argmax-equivalent to the
reference distance).

Per 128-token tile: 4 matmuls fill a [128, 2048] PSUM tile; vector.max
(top-8) + vector.max_index read the scores straight out of PSUM -> the
argmax index per token, plus the top-2 values (used host-side to patch
near-ties against exact f32 scoring). One indirect DMA then gathers the
winning codebook rows for the quantize output. The perplexity scalar is
derived host-side from the histogram of the device-computed indices.
"""

import sys

import numpy as np

for _p in ("/opt/trn_rl_repo",):
    if _p not in sys.path:
        sys.path.insert(0, _p)

N_CORES = 8
N_TOTAL = 64 * 2048          # flattened tokens
N_CORE = N_TOTAL // N_CORES  # 16384
D = 64
K = 2048
TILE = 128
NTILES = N_CORE // TILE      # 128
CHUNK = 512                  # matmul free-dim limit (f32)
NCHUNK = K // CHUNK          # 4

_cache = {}


def _split_excess_waits(nc, limit=1):
    """The walrus build in this image can't encode instructions carrying
    more than ~2 semaphore waits ("Too many sync wait commands"). Hoist
    excess waits onto NoOp instructions inserted just before the offender."""
    import concourse.mybir as mybir

    counter = [0]
    for f in nc.m.functions:
        for b in f.blocks:
            new_insts = []
            for inst in b.instructions:
                si = inst.sync_info
                if si is not None and si.on_wait and len(si.on_wait) > limit:
                    waits = list(si.on_wait)
                    keep = waits[-limit:]
                    extra = waits[:-limit]
                    for w in extra:
                        counter[0] += 1
                        nop = mybir.InstNoOp(
                            name=f"WSPLIT-{counter[0]}", ins=[], outs=[]
                        )
                        nop.engine = inst.engine
                        nop.sync_info = mybir.SyncInfo(on_wait=[w], on_update=[])
                        new_insts.append(nop)
                    si.on_wait = keep
                new_insts.append(inst)
            b.instructions[:] = new_insts
    return counter[0]


def _build_nc(n_core=N_CORE, k=K, split_waits=True):
    import concourse.bass as bass
    import concourse.mybir as mybir
    from concourse.bass import IndirectOffsetOnAxis
    from concourse.tile import TileContext

    F32R = mybir.dt.float32r
    F32 = mybir.dt.float32
    U32 = mybir.dt.uint32

    ntiles = n_core // TILE
    nchunk = k // CHUNK

    nc = bass.Bass()
    xT_d = nc.declare_dram_parameter("xaugt", [D + 1, n_core], F32R, isOutput=False)
    eT_d = nc.declare_dram_parameter("embt", [D + 1, k], F32R, isOutput=False)
    emb_d = nc.declare_dram_parameter("embed", [k, D], F32, isOutput=False)
    quant_d = nc.declare_dram_parameter("quant", [n_core, D], F32, isOutput=True)
    ind8_d = nc.declare_dram_parameter("ind8", [TILE, 8 * ntiles], U32, isOutput=True)
    m8_d = nc.declare_dram_parameter("m8", [TILE, 8 * ntiles], F32, isOutput=True)

    with TileContext(nc) as tc:
        with (
            tc.tile_pool(name="const", bufs=1) as cpool,
            tc.tile_pool(name="keys", bufs=3) as kpool,
            tc.tile_pool(name="persist", bufs=1) as ppool,
            tc.tile_pool(name="ps_sc", bufs=2, space="PSUM") as ps_sc,
        ):
            xT_t = cpool.tile([D + 1, n_core], F32R)
            nc.sync.dma_start(xT_t[:], xT_d[:])
            eT_t = cpool.tile([D + 1, k], F32R)
            nc.sync.dma_start(eT_t[:], eT_d[:])

            ind8_all = ppool.tile([TILE, 8 * ntiles], U32)
            m8_all = ppool.tile([TILE, 8 * ntiles], F32)
            gath = ppool.tile([128, (n_core // 128) * D], F32)

            for it in range(ntiles):
                scores = ps_sc.tile([TILE, k], F32, tag="sc")
                for c in range(nchunk):
                    nc.tensor.matmul(
                        scores[:, c * CHUNK : (c + 1) * CHUNK],
                        xT_t[:, it * TILE : (it + 1) * TILE],
                        eT_t[:, c * CHUNK : (c + 1) * CHUNK],
                        start=True,
                        stop=True,
                    )
                # evacuate scores to SBUF on the (otherwise idle) scalar
                # engine; DVE's max/max_index then read SBUF (cheaper access,
                # PSUM bank freed for the next tile's matmuls sooner)
                keys = kpool.tile([TILE, k], F32, tag="keys")
                nc.scalar.copy(keys[:], scores[:])
                m8 = m8_all[:, 8 * it : 8 * it + 8]
                nc.vector.max(m8, keys[:])
                idx8 = ind8_all[:, 8 * it : 8 * it + 8]
                nc.vector.max_index(idx8, m8, keys[:])
                # gather this tile's codebook rows: gath[p, it*D:(it+1)*D]
                # = embed[idx8[p, 0]]  (one row per partition per call)
                nc.gpsimd.indirect_dma_start(
                    out=gath[:, it * D : (it + 1) * D],
                    out_offset=None,
                    in_=emb_d[:],
                    in_offset=IndirectOffsetOnAxis(ap=idx8[:, 0:1], axis=0),
                )
            nc.sync.dma_start(
                quant_d.rearrange("(it pp) e -> pp it e", pp=128),
                gath[:].rearrange("p (c e) -> p c e", c=n_core // 128),
            )
            nc.sync.dma_start(ind8_d[:], ind8_all[:])
            nc.sync.dma_start(m8_d[:], m8_all[:])
    if split_waits:
        _split_excess_waits(nc)
    nc.finalize()
    return nc


def _get_nc():
    if "nc" not in _cache:
        _cache["nc"] = _build_nc()
    return _cache["nc"]


def _host_prep(x, embed):
    x = np.ascontiguousarray(np.asarray(x), dtype=np.float32)
    embed = np.ascontiguousarray(np.asarray(embed), dtype=np.float32)
    flat = x.reshape(-1, D)
    xaugT = np.empty((D + 1, flat.shape[0]), dtype=np.float32)
    xaugT[:D] = flat.T
    xaugT[D] = 1.0
    eT = np.empty((D + 1, K), dtype=np.float32)
    eT[:D] = 2.0 * embed.T
    eT[D] = -(embed * embed).sum(1)
    return flat, xaugT, eT, embed


def _run(x, embed, trace=False):
    from concourse.bass_utils import run_bass_kernel_spmd

    shape = np.asarray(x).shape
    flat, xaugT, eT, embed = _host_prep(x, embed)

    nc = _get_nc()
    in_maps = [
        {
            "xaugt": np.ascontiguousarray(xaugT[:, i * N_CORE : (i + 1) * N_CORE]),
            "embt": eT,
            "embed": embed,
        }
        for i in range(N_CORES)
    ]
    res = run_bass_kernel_spmd(
        nc, in_maps, core_ids=list(range(N_CORES)), trace=trace
    )

    quants = []
    inds = []
    for i, r in enumerate(res.results):
        ind8 = np.asarray(r["ind8"]).reshape(TILE, NTILES, 8)
        m8v = np.asarray(r["m8"]).reshape(TILE, NTILES, 8)
        # ind in token order: token it*128+pp -> ind8[pp, it, 0]
        ind_i = np.ascontiguousarray(ind8[:, :, 0].T).reshape(-1).view(np.int32).copy()
        # host patch for near-ties: where the device's top-2 scores are too
        # close, recompute the argmax in exact f32 (fp32r / ordering noise)
        gap = m8v[:, :, 0] - m8v[:, :, 1]
        pp, it = np.nonzero(gap < 5e-2)
        if len(pp):
            toks = it * TILE + pp
            fl = flat[i * N_CORE + toks]
            sc = 2.0 * fl @ embed.T - (embed * embed).sum(1)[None, :]
            exact = sc.argmax(1).astype(np.int32)
            ind_flat = ind_i.reshape(NTILES, TILE)
            ind_flat[it, pp] = exact
            ind_i = ind_flat.reshape(-1)
        inds.append(ind_i)
        quant_i = np.asarray(r["quant"])
        if len(pp):
            quant_i = quant_i.copy()
            quant_i[toks] = embed[ind_i[toks]]
        quants.append(quant_i)

    quant = np.concatenate(quants, axis=0)
    ind = np.concatenate(inds, axis=0)

    quantize = quant.reshape(shape)
    embed_ind = ind.reshape(shape[:-1])

    counts = np.bincount(ind, minlength=K).astype(np.float32)
    avg_probs = counts / np.float32(N_TOTAL)
    perplexity = np.exp(
        -np.sum(avg_probs * np.log(avg_probs + np.float32(1e-10)))
    ).astype(np.float32)

    return (quantize, embed_ind, perplexity), res


def kernel(x, embed):
    out, _ = _run(x, embed, trace=False)
    return out


# revision 17
# speedup vs baseline: 4219.8017x; 1.2433x over previous
"""EuclideanCodebook forward (eval) on 8 TRN2 NeuronCores.

Data-parallel: the flattened token dim N=131072 is split across 8 cores
(16384 tokens each); the [2048, 64] codebook is replicated.

Host prep: xaugT = [x.T; ones] (65 x N) and embT = [2*e.T; -|e|^2]
(65 x 2048), so one fp32r matmul per (tile, chunk) yields
scores = 2*x@e.T - |e|^2 directly in PSUM (argmax-equivalent to the
reference distance).

Per 128-token tile: 4 matmuls fill a [128, 2048] PSUM tile; vector.max
(top-8) + vector.max_index read the scores straight out of PSUM -> the
argmax index per token, plus the top-2 values (used host-side to patch
near-ties against exact f32 scoring). One indirect DMA then gathers the
winning codebook rows for the quantize output. The perplexity scalar is
derived host-side from the histogram of the device-computed indices.
"""

import sys

import numpy as np

for _p in ("/opt/trn_rl_repo",):
    if _p not in sys.path:
        sys.path.insert(0, _p)

N_CORES = 8
N_TOTAL = 64 * 2048          # flattened tokens
N_CORE = N_TOTAL // N_CORES  # 16384
D = 64
K = 2048
TILE = 128
NTILES = N_CORE // TILE      # 128
CHUNK = 512                  # matmul free-dim limit (f32)
NCHUNK = K // CHUNK          # 4

_cache = {}


def _split_excess_waits(nc, limit=1):
    """The walrus build in this image can't encode instructions carrying
    more than ~2 semaphore waits ("Too many sync wait commands"). Hoist
    excess waits onto NoOp instructions inserted just before the offender."""
    import concourse.mybir as mybir

    counter = [0]
    for f in nc.m.functions:
        for b in f.blocks:
            new_insts = []
            for inst in b.instructions:
                si = inst.sync_info
                if si is not None and si.on_wait and len(si.on_wait) > limit:
                    waits = list(si.on_wait)
                    keep = waits[-limit:]
                    extra = waits[:-limit]
                    for w in extra:
                        counter[0] += 1
                        nop = mybir.InstNoOp(
                            name=f"WSPLIT-{counter[0]}", ins=[], outs=[]
                        )
                        nop.engine = inst.engine
                        nop.sync_info = mybir.SyncInfo(on_wait=[w], on_update=[])
                        new_insts.append(nop)
                    si.on_wait = keep
                new_insts.append(inst)
            b.instructions[:] = new_insts
    return counter[0]


def _build_nc(n_core=N_CORE, k=K, split_waits=True):
    import concourse.bass as bass
    import concourse.mybir as mybir
    from concourse.bass import IndirectOffsetOnAxis
    from concourse.tile import TileContext

    F32R = mybir.dt.float32r
    F32 = mybir.dt.float32
    U32 = mybir.dt.uint32

    ntiles = n_core // TILE
    nchunk = k // CHUNK

    nc = bass.Bass()
    xT_d = nc.declare_dram_parameter("xaugt", [D + 1, n_core], F32R, isOutput=False)
    eT_d = nc.declare_dram_parameter("embt", [D + 1, k], F32R, isOutput=False)
    emb_d = nc.declare_dram_parameter("embed", [k, D], F32, isOutput=False)
    quant_d = nc.declare_dram_parameter("quant", [n_core, D], F32, isOutput=True)
    ind8_d = nc.declare_dram_parameter("ind8", [TILE, ntiles], U32, isOutput=True)
    m8_d = nc.declare_dram_parameter("m8", [TILE, 8 * ntiles], F32, isOutput=True)

    with TileContext(nc) as tc:
        with (
            tc.tile_pool(name="const", bufs=1) as cpool,
            tc.tile_pool(name="keys", bufs=3) as kpool,
            tc.tile_pool(name="small", bufs=4) as spool,
            tc.tile_pool(name="persist", bufs=1) as ppool,
            tc.tile_pool(name="ps_sc", bufs=2, space="PSUM") as ps_sc,
        ):
            xT_t = cpool.tile([D + 1, n_core], F32R)
            nc.sync.dma_start(xT_t[:], xT_d[:])
            eT_t = cpool.tile([D + 1, k], F32R)
            nc.sync.dma_start(eT_t[:], eT_d[:])

            half = k // 2
            m8_all = ppool.tile([TILE, 8 * ntiles], F32)
            indu_all = ppool.tile([TILE, ntiles], U32)
            gath = ppool.tile([128, (n_core // 128) * D], F32)

            for it in range(ntiles):
                scores = ps_sc.tile([TILE, k], F32, tag="sc")
                for c in range(nchunk):
                    nc.tensor.matmul(
                        scores[:, c * CHUNK : (c + 1) * CHUNK],
                        xT_t[:, it * TILE : (it + 1) * TILE],
                        eT_t[:, c * CHUNK : (c + 1) * CHUNK],
                        start=True,
                        stop=True,
                    )
                # ScalarE evacuates the upper K-half to SBUF; DVE folds the
                # two halves elementwise (max) so max/max_index scan only
                # k/2 elements. The winning half-bit is recovered from a
                # ScalarE sign-count of exact matches in the upper half.
                keys_hi = kpool.tile([TILE, half], F32, tag="keys")
                nc.scalar.copy(keys_hi[:], scores[:, half:])
                f1 = kpool.tile([TILE, half], F32, tag="fold")
                nc.vector.tensor_tensor(
                    f1[:], scores[:, :half], keys_hi[:], mybir.AluOpType.max
                )
                m8 = m8_all[:, 8 * it : 8 * it + 8]
                nc.vector.max(m8, f1[:])
                # acc = sum(sign(M - keys_hi)) = half - #{keys_hi == M}
                junk = kpool.tile([TILE, half], F32, tag="junk")
                acc = spool.tile([TILE, 1], F32, tag="acc")
                nc.scalar.activation(
                    junk[:],
                    keys_hi[:],
                    mybir.ActivationFunctionType.Sign,
                    bias=m8[:, 0:1],
                    scale=-1.0,
                    accum_out=acc[:],
                )
                r8 = spool.tile([TILE, 8], U32, tag="r8")
                nc.vector.max_index(r8[:], m8, f1[:])
                # idx = b1*half + r,  b1 = half - acc
                t1 = spool.tile([TILE, 1], F32, tag="t1")
                nc.vector.tensor_scalar(
                    t1[:], acc[:], -float(half), float(half * half),
                    mybir.AluOpType.mult, mybir.AluOpType.add,
                )
                rf = spool.tile([TILE, 1], F32, tag="rf")
                nc.vector.tensor_copy(rf[:], r8[:, 0:1])
                idxf = spool.tile([TILE, 1], F32, tag="idxf")
                nc.vector.tensor_tensor(
                    idxf[:], t1[:], rf[:], mybir.AluOpType.add
                )
                ind_col = indu_all[:, it : it + 1]
                nc.vector.tensor_copy(ind_col, idxf[:])
                # gather this tile's codebook rows: gath[p, it*D:(it+1)*D]
                # = embed[ind_col[p]]  (one row per partition per call)
                nc.gpsimd.indirect_dma_start(
                    out=gath[:, it * D : (it + 1) * D],
                    out_offset=None,
                    in_=emb_d[:],
                    in_offset=IndirectOffsetOnAxis(ap=ind_col, axis=0),
                )
            nc.sync.dma_start(
                quant_d.rearrange("(it pp) e -> pp it e", pp=128),
                gath[:].rearrange("p (c e) -> p c e", c=n_core // 128),
            )
            nc.sync.dma_start(ind8_d[:], indu_all[:])
            nc.sync.dma_start(m8_d[:], m8_all[:])
    if split_waits:
        _split_excess_waits(nc)
    nc.finalize()
    return nc


def _get_nc():
    if "nc" not in _cache:
        _cache["nc"] = _build_nc()
    return _cache["nc"]


def _host_prep(x, embed):
    x = np.ascontiguousarray(np.asarray(x), dtype=np.float32)
    embed = np.ascontiguousarray(np.asarray(embed), dtype=np.float32)
    flat = x.reshape(-1, D)
    xaugT = np.empty((D + 1, flat.shape[0]), dtype=np.float32)
    xaugT[:D] = flat.T
    xaugT[D] = 1.0
    eT = np.empty((D + 1, K), dtype=np.float32)
    eT[:D] = 2.0 * embed.T
    eT[D] = -(embed * embed).sum(1)
    return flat, xaugT, eT, embed


def _run(x, embed, trace=False):
    from concourse.bass_utils import run_bass_kernel_spmd

    shape = np.asarray(x).shape
    flat, xaugT, eT, embed = _host_prep(x, embed)

    nc = _get_nc()
    in_maps = [
        {
            "xaugt": np.ascontiguousarray(xaugT[:, i * N_CORE : (i + 1) * N_CORE]),
            "embt": eT,
            "embed": embed,
        }
        for i in range(N_CORES)
    ]
    res = run_bass_kernel_spmd(
        nc, in_maps, core_ids=list(range(N_CORES)), trace=trace
    )

    quants = []
    inds = []
    for i, r in enumerate(res.results):
        indu = np.asarray(r["ind8"])  # [TILE, NTILES] u32
        m8v = np.asarray(r["m8"]).reshape(TILE, NTILES, 8)
        # ind in token order: token it*128+pp -> indu[pp, it]
        ind_i = np.ascontiguousarray(indu.T).reshape(-1).view(np.int32).copy()
        # host patch for near-ties: where the device's top-2 scores are too
        # close, recompute the argmax in exact f32 (fp32r / ordering noise)
        gap = m8v[:, :, 0] - m8v[:, :, 1]
        pp, it = np.nonzero(gap < 5e-2)
        if len(pp):
            toks = it * TILE + pp
            fl = flat[i * N_CORE + toks]
            sc = 2.0 * fl @ embed.T - (embed * embed).sum(1)[None, :]
            exact = sc.argmax(1).astype(np.int32)
            ind_flat = ind_i.reshape(NTILES, TILE)
            ind_flat[it, pp] = exact
            ind_i = ind_flat.reshape(-1)
        inds.append(ind_i)
        quant_i = np.asarray(r["quant"])
        if len(pp):
            quant_i = quant_i.copy()
            quant_i[toks] = embed[ind_i[toks]]
        quants.append(quant_i)

    quant = np.concatenate(quants, axis=0)
    ind = np.concatenate(inds, axis=0)

    quantize = quant.reshape(shape)
    embed_ind = ind.reshape(shape[:-1])

    counts = np.bincount(ind, minlength=K).astype(np.float32)
    avg_probs = counts / np.float32(N_TOTAL)
    perplexity = np.exp(
        -np.sum(avg_probs * np.log(avg_probs + np.float32(1e-10)))
    ).astype(np.float32)

    return (quantize, embed_ind, perplexity), res


def kernel(x, embed):
    out, _ = _run(x, embed, trace=False)
    return out


# revision 18
# speedup vs baseline: 4597.3601x; 1.0895x over previous
"""EuclideanCodebook forward (eval) on 8 TRN2 NeuronCores.

Data-parallel: the flattened token dim N=131072 is split across 8 cores
(16384 tokens each); the [2048, 64] codebook is replicated.

Host prep: xaugT = [x.T; ones] (65 x N) and embT = [2*e.T; -|e|^2]
(65 x 2048), so one fp32r matmul per (tile, chunk) yields
scores = 2*x@e.T - |e|^2 directly in PSUM (argmax-equivalent to the
reference distance).

Per 128-token tile: 4 matmuls fill a [128, 2048] PSUM tile; vector.max
(top-8) + vector.max_index read the scores straight out of PSUM -> the
argmax index per token, plus the top-2 values (used host-side to patch
near-ties against exact f32 scoring). One indirect DMA then gathers the
winning codebook rows for the quantize output. The perplexity scalar is
derived host-side from the histogram of the device-computed indices.
"""

import sys

import numpy as np

for _p in ("/opt/trn_rl_repo",):
    if _p not in sys.path:
        sys.path.insert(0, _p)

N_CORES = 8
N_TOTAL = 64 * 2048          # flattened tokens
N_CORE = N_TOTAL // N_CORES  # 16384
D = 64
K = 2048
TILE = 128
NTILES = N_CORE // TILE      # 128
CHUNK = 512                  # matmul free-dim limit (f32)
NCHUNK = K // CHUNK          # 4

_cache = {}


def _split_excess_waits(nc, limit=1):
    """The walrus build in this image can't encode instructions carrying
    more than ~2 semaphore waits ("Too many sync wait commands"). Hoist
    excess waits onto NoOp instructions inserted just before the offender."""
    import concourse.mybir as mybir

    counter = [0]
    for f in nc.m.functions:
        for b in f.blocks:
            new_insts = []
            for inst in b.instructions:
                si = inst.sync_info
                if si is not None and si.on_wait and len(si.on_wait) > limit:
                    waits = list(si.on_wait)
                    keep = waits[-limit:]
                    extra = waits[:-limit]
                    for w in extra:
                        counter[0] += 1
                        nop = mybir.InstNoOp(
                            name=f"WSPLIT-{counter[0]}", ins=[], outs=[]
                        )
                        nop.engine = inst.engine
                        nop.sync_info = mybir.SyncInfo(on_wait=[w], on_update=[])
                        new_insts.append(nop)
                    si.on_wait = keep
                new_insts.append(inst)
            b.instructions[:] = new_insts
    return counter[0]


def _build_nc(n_core=N_CORE, k=K, split_waits=True):
    import concourse.bass as bass
    import concourse.mybir as mybir
    from concourse.bass import IndirectOffsetOnAxis
    from concourse.tile import TileContext

    F32R = mybir.dt.float32r
    F32 = mybir.dt.float32
    U32 = mybir.dt.uint32

    ntiles = n_core // TILE
    nchunk = k // CHUNK

    nc = bass.Bass()
    xT_d = nc.declare_dram_parameter("xaugt", [D + 1, n_core], F32R, isOutput=False)
    eT_d = nc.declare_dram_parameter("embt", [D + 1, k], F32R, isOutput=False)
    emb_d = nc.declare_dram_parameter("embed", [k, D], F32, isOutput=False)
    quant_d = nc.declare_dram_parameter("quant", [n_core, D], F32, isOutput=True)
    ind8_d = nc.declare_dram_parameter("ind8", [TILE, ntiles], U32, isOutput=True)
    m8_d = nc.declare_dram_parameter("m8", [TILE, 8 * ntiles], F32, isOutput=True)

    with TileContext(nc) as tc:
        with (
            tc.tile_pool(name="const", bufs=1) as cpool,
            tc.tile_pool(name="keys", bufs=3) as kpool,
            tc.tile_pool(name="small", bufs=4) as spool,
            tc.tile_pool(name="persist", bufs=1) as ppool,
            tc.tile_pool(name="ps_sc", bufs=2, space="PSUM") as ps_sc,
        ):
            xT_t = cpool.tile([D + 1, n_core], F32R)
            nc.sync.dma_start(xT_t[:], xT_d[:])
            eT_t = cpool.tile([D + 1, k], F32R)
            nc.sync.dma_start(eT_t[:], eT_d[:])

            half = k // 2
            m8_all = ppool.tile([TILE, 8 * ntiles], F32)
            indu_all = ppool.tile([TILE, ntiles], U32)
            gath = ppool.tile([128, (n_core // 128) * D], F32)

            for it in range(ntiles):
                scores = ps_sc.tile([TILE, k], F32, tag="sc")
                for c in range(nchunk):
                    nc.tensor.matmul(
                        scores[:, c * CHUNK : (c + 1) * CHUNK],
                        xT_t[:, it * TILE : (it + 1) * TILE],
                        eT_t[:, c * CHUNK : (c + 1) * CHUNK],
                        start=True,
                        stop=True,
                    )
                # ScalarE evacuates the upper K-half to SBUF; DVE folds the
                # two halves elementwise (max) so max/max_index scan only
                # k/2 elements. The winning half-bit is recovered from a
                # ScalarE sign-count of exact matches in the upper half.
                keys_hi = kpool.tile([TILE, half], F32, tag="keys")
                nc.scalar.copy(keys_hi[:], scores[:, half:])
                f1 = kpool.tile([TILE, half], F32, tag="fold")
                nc.vector.tensor_tensor(
                    f1[:], scores[:, :half], keys_hi[:], mybir.AluOpType.max
                )
                quart = half // 2
                f2 = kpool.tile([TILE, quart], F32, tag="fold2")
                nc.vector.tensor_tensor(
                    f2[:], f1[:, :quart], f1[:, quart:], mybir.AluOpType.max
                )
                m8 = m8_all[:, 8 * it : 8 * it + 8]
                nc.vector.max(m8, f2[:])
                # acc1 = sum(sign(M - keys_hi)) = half - #{keys_hi == M}
                junk = kpool.tile([TILE, half], F32, tag="junk")
                acc1 = spool.tile([TILE, 1], F32, tag="acc1")
                nc.scalar.activation(
                    junk[:],
                    keys_hi[:],
                    mybir.ActivationFunctionType.Sign,
                    bias=m8[:, 0:1],
                    scale=-1.0,
                    accum_out=acc1[:],
                )
                # acc2 = quart - #{f1[quart:] == M}  (winner's quarter bit)
                junk2 = kpool.tile([TILE, quart], F32, tag="junk2")
                acc2 = spool.tile([TILE, 1], F32, tag="acc2")
                nc.scalar.activation(
                    junk2[:],
                    f1[:, quart:],
                    mybir.ActivationFunctionType.Sign,
                    bias=m8[:, 0:1],
                    scale=-1.0,
                    accum_out=acc2[:],
                )
                r8 = spool.tile([TILE, 8], U32, tag="r8")
                nc.vector.max_index(r8[:], m8, f2[:])
                # idx = b1*half + b0*quart + r, b1 = half-acc1, b0 = quart-acc2
                t1 = spool.tile([TILE, 1], F32, tag="t1")
                nc.vector.tensor_scalar(
                    t1[:], acc1[:], -float(half),
                    float(half * half + quart * quart),
                    mybir.AluOpType.mult, mybir.AluOpType.add,
                )
                t2 = spool.tile([TILE, 1], F32, tag="t2")
                nc.vector.tensor_scalar(
                    t2[:], acc2[:], -float(quart), None,
                    mybir.AluOpType.mult,
                )
                t3 = spool.tile([TILE, 1], F32, tag="t3")
                nc.vector.tensor_tensor(
                    t3[:], t1[:], t2[:], mybir.AluOpType.add
                )
                rf = spool.tile([TILE, 1], F32, tag="rf")
                nc.vector.tensor_copy(rf[:], r8[:, 0:1])
                idxf = spool.tile([TILE, 1], F32, tag="idxf")
                nc.vector.tensor_tensor(
                    idxf[:], t3[:], rf[:], mybir.AluOpType.add
                )
                ind_col = indu_all[:, it : it + 1]
                nc.vector.tensor_copy(ind_col, idxf[:])
                # gather this tile's codebook rows: gath[p, it*D:(it+1)*D]
                # = embed[ind_col[p]]  (one row per partition per call)
                nc.gpsimd.indirect_dma_start(
                    out=gath[:, it * D : (it + 1) * D],
                    out_offset=None,
                    in_=emb_d[:],
                    in_offset=IndirectOffsetOnAxis(ap=ind_col, axis=0),
                )
            nc.sync.dma_start(
                quant_d.rearrange("(it pp) e -> pp it e", pp=128),
                gath[:].rearrange("p (c e) -> p c e", c=n_core // 128),
            )
            nc.sync.dma_start(ind8_d[:], indu_all[:])
            nc.sync.dma_start(m8_d[:], m8_all[:])
    if split_waits:
        _split_excess_waits(nc)
    nc.finalize()
    return nc


def _get_nc():
    if "nc" not in _cache:
        _cache["nc"] = _build_nc()
    return _cache["nc"]


def _host_prep(x, embed):
    x = np.ascontiguousarray(np.asarray(x), dtype=np.float32)
    embed = np.ascontiguousarray(np.asarray(embed), dtype=np.float32)
    flat = x.reshape(-1, D)
    xaugT = np.empty((D + 1, flat.shape[0]), dtype=np.float32)
    xaugT[:D] = flat.T
    xaugT[D] = 1.0
    eT = np.empty((D + 1, K), dtype=np.float32)
    eT[:D] = 2.0 * embed.T
    eT[D] = -(embed * embed).sum(1)
    return flat, xaugT, eT, embed


def _run(x, embed, trace=False):
    from concourse.bass_utils import run_bass_kernel_spmd

    shape = np.asarray(x).shape
    flat, xaugT, eT, embed = _host_prep(x, embed)

    nc = _get_nc()
    in_maps = [
        {
            "xaugt": np.ascontiguousarray(xaugT[:, i * N_CORE : (i + 1) * N_CORE]),
            "embt": eT,
            "embed": embed,
        }
        for i in range(N_CORES)
    ]
    res = run_bass_kernel_spmd(
        nc, in_maps, core_ids=list(range(N_CORES)), trace=trace
    )

    quants = []
    inds = []
    for i, r in enumerate(res.results):
        indu = np.asarray(r["ind8"])  # [TILE, NTILES] u32
        m8v = np.asarray(r["m8"]).reshape(TILE, NTILES, 8)
        # ind in token order: token it*128+pp -> indu[pp, it]
        ind_i = np.ascontiguousarray(indu.T).reshape(-1).view(np.int32).copy()
        # host patch for near-ties: where the device's top-2 scores are too
        # close, recompute the argmax in exact f32 (fp32r / ordering noise)
        gap = m8v[:, :, 0] - m8v[:, :, 1]
        pp, it = np.nonzero(gap < 5e-2)
        if len(pp):
            toks = it * TILE + pp
            fl = flat[i * N_CORE + toks]
            sc = 2.0 * fl @ embed.T - (embed * embed).sum(1)[None, :]
            exact = sc.argmax(1).astype(np.int32)
            ind_flat = ind_i.reshape(NTILES, TILE)
            ind_flat[it, pp] = exact
            ind_i = ind_flat.reshape(-1)
        inds.append(ind_i)
        quant_i = np.asarray(r["quant"])
        if len(pp):
            quant_i = quant_i.copy()
            quant_i[toks] = embed[ind_i[toks]]
        quants.append(quant_i)

    quant = np.concatenate(quants, axis=0)
    ind = np.concatenate(inds, axis=0)

    quantize = quant.reshape(shape)
    embed_ind = ind.reshape(shape[:-1])

    counts = np.bincount(ind, minlength=K).astype(np.float32)
    avg_probs = counts / np.float32(N_TOTAL)
    perplexity = np.exp(
        -np.sum(avg_probs * np.log(avg_probs + np.float32(1e-10)))
    ).astype(np.float32)

    return (quantize, embed_ind, perplexity), res


def kernel(x, embed):
    out, _ = _run(x, embed, trace=False)
    return out
